# revision 31
# baseline (speedup 1.0000x reference)
"""DILATE loss (soft-DTW shape + temporal) on 8 Trainium2 NeuronCores.

Strategy: central finite difference + bidirectional DP.

gamma=0.01 makes the soft-DTW effectively a hard min-plus (Viterbi) DP,
and the temporal term is
  sum(E * Omega) = d/d(eps) softdtw(D + eps*Omega)  at eps=0
since E = d(softdtw)/dD. Each series is solved on D + eps*Omega and on
D - eps*Omega and the host combines:
  vals = (A + B) / 2                  (shape term)
  sum(E*Omega) = (A - B) / (2 eps)    (temporal term)

The DP itself is split at the middle row: a forward chain from (0,0)
and a suffix chain from (N,N) — the latter is the same forward
recurrence on the index-reversed cost matrix, and Omega is
reversal-invariant. Both chains for both FD signs run in ONE
64-row instruction stream over 128 SBUF partitions
(32 series x {+eps,-eps} x {fwd,bwd}); per-op cost on trn2 depends
only on free-dim size, so the extra partitions are time-free while the
serial DP halves. The halves meet between rows 64 and 65:
  total = min_j ( Mf[64,j] + min(num[65,j], num[65,j+1]) )
with num[65,j] = Br[64, N+1-j]; this 128-element reduction joins the
scalar loss reduction on the host.

Engines: ACT builds D rows fused as Square(-o_j + t_i) (bias = t_i per
partition); Pool folds +-eps*Omega in behind it; DVE runs the serial
chain 64 x (TT-min fp16-2x + scan(min,add)). The intra-chain DVE edges
are relaxed to nosync (queue-order) so rows chain back-to-back. M rows
are stored fp16 (scan state is fp32 internally); the final row of each
chain is written fp32 so the corner values the FD differences keep full
precision.
"""
import sys
if "/opt/trn_rl_repo" not in sys.path:
    sys.path.insert(0, "/opt/trn_rl_repo")
import numpy as np
from contextlib import ExitStack

import concourse.bass as bass
import concourse.bacc as bacc
import concourse.mybir as mybir
import concourse.tile as tile
from concourse.mybir import AluOpType, ActivationFunctionType

F32 = mybir.dt.float32
F16 = mybir.dt.float16
S = 32          # series per core
SP = 128        # partitions: series x {+eps,-eps} x {fwd,bwd}
N = 128         # DP size (= T)
H = N // 2      # rows per chain
RS = N + 1      # M-table row stride (col 0 = boundary)
BIG = 1e30
EPS = 3e-5      # FD step on the Omega perturbation
N_CORES = 8


def ap(t, off, dims):
    base = t[:]
    return bass.AP(base.tensor, base.offset + off, [base.ap[0]] + dims)


def _build_kernel():
    nc = bacc.Bacc("TRN2", target_bir_lowering=False, debug=False)
    to_d = nc.dram_tensor("to", [SP, 2 * N], F32, kind="ExternalInput")
    v_d = nc.dram_tensor("v", [SP, H * N], F16, kind="ExternalInput")
    vals_d = nc.dram_tensor("vals", [SP, RS], F32, kind="ExternalOutput")

    with tile.TileContext(nc) as tc, ExitStack() as ctx:
        pool = ctx.enter_context(tc.tile_pool(name="main", bufs=1))
        to_s = pool.tile([SP, 2 * N], F32, tag="to_s")   # [t | o] (bwd half reversed)
        v_s = pool.tile([SP, H * N], F16, tag="v_s")
        D_s = pool.tile([SP, H * N], F32, tag="D_s")
        M_s = pool.tile([SP, H * RS], F16, tag="M_s")    # rows 0..H-1
        Ml_s = pool.tile([SP, RS], F32, tag="Ml_s")      # row H, fp32
        ent_s = pool.tile([SP, N], F16, tag="ent_s")
        scr_s = pool.tile([SP, 1], F32, tag="scr_s")

        # dummy activation on scratch: hoists the Square table load (1.28us)
        # to t=0 so it overlaps the input DMAs instead of stalling row 1
        nc.scalar.activation(scr_s[:], scr_s[:],
                             ActivationFunctionType.Square)

        # [o | t] layout, split input DMA: the first transfer carries o plus
        # the first 32 bias columns (608B/partition instead of 1KB) so ACT's
        # first D row starts earlier; the t-tail goes out on the otherwise
        # idle ACT queue so the SP queue order (and the eps*Omega chunk
        # timing behind it) is untouched.
        CUT = N + 32
        nc.sync.dma_start(
            ap(to_s, 0, [[1, CUT]]),
            bass.AP(to_d, 0, [[2 * N, SP], [1, CUT]]))
        nc.scalar.dma_start(
            ap(to_s, CUT, [[1, 2 * N - CUT]]),
            bass.AP(to_d, CUT, [[2 * N, SP], [1, 2 * N - CUT]]))
        # eps*Omega chunks: first ones small so the build pipeline starts fast
        vch = [2, 2, 4, 8, 16, 32]
        r0 = 0
        for cn in vch:
            nc.sync.dma_start(
                ap(v_s, r0 * N, [[1, cn * N]]),
                bass.AP(v_d, r0 * N, [[H * N, SP], [1, cn * N]]))
            r0 += cn

        # M boundary: row 0 = BIG except M[0,0] = 0; col 0 of rows 1..H-1 =
        # BIG. The fp16 sentinel is 30000: far above any real path cost
        # (< ~1000) and never accumulated (every sentinel loses its min
        # against a real value within one step of the boundary).
        BIG16 = 30000.0
        nc.gpsimd.memset(ap(M_s, 0, [[1, RS]]), BIG16)
        nc.gpsimd.memset(ap(M_s, 0, [[1, 1]]), 0.0)
        nc.gpsimd.memset(ap(M_s, RS, [[RS, H - 1], [1, 1]]), BIG16)

        def m_off(r):
            return r * RS

        # D build: ACT fuses (t_i - o_j)^2 per row; +-eps*Omega is folded in
        # behind it. The first DVE_ADD_ROWS rows are added on DVE itself
        # via scan(bypass, add) (DVE is idle during startup and this keeps
        # Pool's sem hops off the early critical path); the rest go to Pool
        # as plain TT adds (TensorScalarPtr is DVE-only in the real ISA) in
        # small chunks — the DP consumes rows at ~321ns while ACT produces
        # at ~292ns and Pool adds ~254ns/row, so chunks must stay small
        # enough that chunk [a..b) completes before the DP reaches row a.
        DVE_ADD_ROWS = 7
        dve_adds = []
        bounds = [DVE_ADD_ROWS]
        while bounds[-1] < H:
            a = bounds[-1]
            cn = 2 if a < 13 else 4
            bounds.append(min(H, a + cn))
        o_ap = ap(to_s, 0, [[1, N]])
        for i in range(DVE_ADD_ROWS):
            nc.scalar.activation(
                ap(D_s, i * N, [[1, N]]), o_ap,
                ActivationFunctionType.Square,
                bias=ap(to_s, N + i, [[1, 1]]), scale=-1.0)
            dch = ap(D_s, i * N, [[1, N]])
            dve_adds.append(nc.vector.tensor_tensor_scan(
                dch, dch, ap(v_s, i * N, [[1, N]]),
                0.0, AluOpType.bypass, AluOpType.add))
        for a, b in zip(bounds[:-1], bounds[1:]):
            for i in range(a, b):
                nc.scalar.activation(
                    ap(D_s, i * N, [[1, N]]), o_ap,
                    ActivationFunctionType.Square,
                    bias=ap(to_s, N + i, [[1, 1]]), scale=-1.0)
            dch = ap(D_s, a * N, [[1, (b - a) * N]])
            nc.gpsimd.tensor_tensor(
                dch, dch, ap(v_s, a * N, [[1, (b - a) * N]]),
                AluOpType.add)

        # forward min-plus DP: M[r,j] = D[r,j] + min(M[r-1,j-1], M[r-1,j], M[r,j-1])
        dp_insts = []
        for r in range(1, H + 1):
            dp_insts.append(nc.vector.tensor_tensor(
                ent_s[:],
                ap(M_s, m_off(r - 1), [[1, N]]),
                ap(M_s, m_off(r - 1) + 1, [[1, N]]),
                AluOpType.min))
            out_row = (ap(M_s, m_off(r) + 1, [[1, N]]) if r < H
                       else ap(Ml_s, 1, [[1, N]]))
            dp_insts.append(nc.vector.tensor_tensor_scan(
                out_row,
                ent_s[:],
                ap(D_s, (r - 1) * N, [[1, N]]),
                BIG, AluOpType.min, AluOpType.add))

        # The DP is one serial chain of same-engine (DVE) ops with
        # ascending same-shape access; engine in-order execution plus the
        # 128-cycle op length covers the SBUF write-ack pipeline, so the
        # intra-chain edges don't need runtime semaphores. Relax them to
        # nosync (queue-order) edges — without this every row pays two
        # ~90ns sem round-trips plus a SEQ-blocking EventSemaphore wait
        # (~290ns/row on the critical path).
        import bass_rust as _br
        _NOSYNC = _br.DependencyInfo.NO_SYNC_ONLY
        dp_insts = dve_adds + dp_insts
        dp_names = {bi.ins.name for bi in dp_insts}
        for bi in dp_insts:
            inst = bi.ins
            for name, info in inst.dependency_edges():
                if name in dp_names:
                    inst.remove_dependency(name)
                    inst.add_dependency(name, _NOSYNC)

        nc.sync.dma_start(vals_d.ap(), Ml_s[:])

    nc.compile()
    return nc


_NC_CACHE = None


def _get_nc():
    global _NC_CACHE
    if _NC_CACHE is None:
        _NC_CACHE = _build_kernel()
    return _NC_CACHE


def _v_const():
    """[SP, H*N] fp16 +-eps*Omega rows 1..H; Omega is reversal-invariant so
    the bwd half uses the same values. Sign per 32-block: [+, -, +, -]."""
    idx = np.arange(N, dtype=np.float64)
    om = ((idx[:, None] - idx[None, :]) ** 2)[:H].reshape(-1)
    v = (EPS * om).astype(np.float16)
    return np.concatenate([
        np.broadcast_to(v, (S, H * N)),
        np.broadcast_to(-v, (S, H * N)),
        np.broadcast_to(v, (S, H * N)),
        np.broadcast_to(-v, (S, H * N)),
    ]).astype(np.float16)


_EXEC_CACHE = None


def _get_exec():
    """Build the sharded jitted executable once (mirrors bass2jax's
    run_bass_via_pjrt multi-core path) and keep the constant v input
    resident on the devices."""
    global _EXEC_CACHE
    if _EXEC_CACHE is not None:
        return _EXEC_CACHE
    import jax
    import concourse.mybir as _mybir
    from jax.sharding import Mesh, PartitionSpec, NamedSharding
    from jax.experimental.shard_map import shard_map
    from concourse.bass2jax import (
        _bass_exec_p, install_neuronx_cc_hook, partition_id_tensor)

    nc = _get_nc()
    install_neuronx_cc_hook()
    partition_name = nc.partition_id_tensor.name if nc.partition_id_tensor else None
    in_names, out_names, out_avals, zero_outs = [], [], [], []
    for alloc in nc.m.functions[0].allocations:
        if not isinstance(alloc, _mybir.MemoryLocationSet):
            continue
        name = alloc.memorylocations[0].name
        if alloc.kind == "ExternalInput":
            if name != partition_name:
                in_names.append(name)
        elif alloc.kind == "ExternalOutput":
            shape = tuple(alloc.tensor_shape)
            dtype = _mybir.dt.np(alloc.dtype)
            out_names.append(name)
            out_avals.append(jax.core.ShapedArray(shape, dtype))
            zero_outs.append(np.zeros(shape, dtype))
    n_params = len(in_names)
    all_in_names = list(in_names) + list(out_names)
    if partition_name is not None:
        all_in_names.append(partition_name)
    donate = tuple(range(n_params, n_params + len(out_names)))

    def _body(*args):
        operands = list(args)
        if partition_name is not None:
            operands.append(partition_id_tensor())
        return tuple(_bass_exec_p.bind(
            *operands,
            out_avals=tuple(out_avals),
            in_names=tuple(all_in_names),
            out_names=tuple(out_names),
            lowering_input_output_aliases=(),
            sim_require_finite=True,
            sim_require_nnan=True,
            nc=nc,
        ))

    devices = jax.devices()[:N_CORES]
    mesh = Mesh(np.asarray(devices), ("core",))
    in_specs = (PartitionSpec("core"),) * (n_params + len(out_names))
    out_specs = (PartitionSpec("core"),) * len(out_names)
    sharded = jax.jit(
        shard_map(_body, mesh=mesh, in_specs=in_specs, out_specs=out_specs,
                  check_rep=False),
        donate_argnums=donate, keep_unused=True)
    shard = NamedSharding(mesh, PartitionSpec("core"))
    v_dev = jax.device_put(
        np.concatenate([_v_const()] * N_CORES, axis=0), shard)
    _EXEC_CACHE = (sharded, in_names, out_names, zero_outs, shard, v_dev)
    return _EXEC_CACHE


def kernel(outputs, targets):
    """outputs, targets: [64, 128, 4] float32 -> scalar float32 loss."""
    sharded, in_names, out_names, zero_outs, shard, v_dev = _get_exec()
    outputs = np.asarray(outputs, np.float32)
    targets = np.asarray(targets, np.float32)
    B, T, C = outputs.shape
    t = np.transpose(targets, (0, 2, 1)).reshape(N_CORES, S, T)
    o = np.transpose(outputs, (0, 2, 1)).reshape(N_CORES, S, T)
    # per core, 128 partitions: [fwd A | fwd B | bwd A | bwd B];
    # free dim = [t | o], with the bwd halves index-reversed
    fwd = np.concatenate([o, t], axis=2)                        # [8, 32, 2T]
    bwd = np.concatenate([o[:, :, ::-1], t[:, :, ::-1]], axis=2)
    to = np.concatenate([fwd, fwd, bwd, bwd], axis=1)           # [8, 128, 2T]
    to2 = np.ascontiguousarray(to.reshape(N_CORES * SP, 2 * T))
    by_name = {"to": to2, "v": v_dev}
    concat_in = [by_name[name] for name in in_names]
    concat_zeros = [
        np.zeros((N_CORES * z.shape[0], *z.shape[1:]), z.dtype) for z in zero_outs
    ]
    out_arrs = sharded(*concat_in, *concat_zeros)
    outs = {name: np.asarray(out_arrs[i]) for i, name in enumerate(out_names)}
    rows = outs["vals"].reshape(N_CORES, 4, S, RS).astype(np.float64)
    # meeting-point merge between rows H and H+1:
    #   total = min_j ( Mf[H,j] + min(num[H+1,j], num[H+1,j+1]) )
    #   num[H+1,j] = Br[H, N+1-j]
    j = np.arange(1, N + 1)
    BIGF = np.float64(BIG)
    res = {}
    for h, (fi, bi) in enumerate(((0, 2), (1, 3))):
        Mf, Br = rows[:, fi], rows[:, bi]                       # [8, 32, RS]
        numj = Br[:, :, N + 1 - j]
        numj1 = np.where((N - j) >= 1, Br[:, :, np.clip(N - j, 0, N)], BIGF)
        res[h] = (Mf[:, :, 1:] + np.minimum(numj, numj1)).min(axis=2)
    A, Bm = res[0], res[1]
    vals = (A + Bm) / 2.0
    s_fd = (A - Bm) / (2.0 * EPS)
    loss = 0.5 * (vals.sum() / B) + 0.5 * (s_fd.sum() / (B * T * T))
    return np.float32(loss)


# revision 32
# speedup vs baseline: 1.0246x; 1.0246x over previous
"""DILATE loss (soft-DTW shape + temporal) on 8 Trainium2 NeuronCores.

Strategy: central finite difference + bidirectional DP.

gamma=0.01 makes the soft-DTW effectively a hard min-plus (Viterbi) DP,
and the temporal term is
  sum(E * Omega) = d/d(eps) softdtw(D + eps*Omega)  at eps=0
since E = d(softdtw)/dD. Each series is solved on D + eps*Omega and on
D - eps*Omega and the host combines:
  vals = (A + B) / 2                  (shape term)
  sum(E*Omega) = (A - B) / (2 eps)    (temporal term)

The DP itself is split at the middle row: a forward chain from (0,0)
and a suffix chain from (N,N) — the latter is the same forward
recurrence on the index-reversed cost matrix, and Omega is
reversal-invariant. Both chains for both FD signs run in ONE
64-row instruction stream over 128 SBUF partitions
(32 series x {+eps,-eps} x {fwd,bwd}); per-op cost on trn2 depends
only on free-dim size, so the extra partitions are time-free while the
serial DP halves. The halves meet between rows 64 and 65:
  total = min_j ( Mf[64,j] + min(num[65,j], num[65,j+1]) )
with num[65,j] = Br[64, N+1-j]; this 128-element reduction joins the
scalar loss reduction on the host.

Engines: ACT builds D rows fused as Square(-o_j + t_i) (bias = t_i per
partition); Pool folds +-eps*Omega in behind it; DVE runs the serial
chain 64 x (TT-min fp16-2x + scan(min,add)). The intra-chain DVE edges
are relaxed to nosync (queue-order) so rows chain back-to-back. M rows
are stored fp16 (scan state is fp32 internally); the final row of each
chain is written fp32 so the corner values the FD differences keep full
precision.
"""
import sys
if "/opt/trn_rl_repo" not in sys.path:
    sys.path.insert(0, "/opt/trn_rl_repo")
import numpy as np
from contextlib import ExitStack

import concourse.bass as bass
import concourse.bacc as bacc
import concourse.mybir as mybir
import concourse.tile as tile
from concourse.mybir import AluOpType, ActivationFunctionType

F32 = mybir.dt.float32
F16 = mybir.dt.float16
S = 32          # series per core
SP = 128        # partitions: series x {+eps,-eps} x {fwd,bwd}
N = 128         # DP size (= T)
H = N // 2      # rows per chain
RS = N + 1      # M-table row stride (col 0 = boundary)
BIG = 1e30
EPS = 3e-5      # FD step on the Omega perturbation
N_CORES = 8


def ap(t, off, dims):
    base = t[:]
    return bass.AP(base.tensor, base.offset + off, [base.ap[0]] + dims)


def _build_kernel():
    nc = bacc.Bacc("TRN2", target_bir_lowering=False, debug=False)
    to_d = nc.dram_tensor("to", [SP, 2 * N], F32, kind="ExternalInput")
    v_d = nc.dram_tensor("v", [SP, H * N], F16, kind="ExternalInput")
    vals_d = nc.dram_tensor("vals", [SP, RS], F32, kind="ExternalOutput")

    with tile.TileContext(nc) as tc, ExitStack() as ctx:
        pool = ctx.enter_context(tc.tile_pool(name="main", bufs=1))
        to_s = pool.tile([SP, 2 * N], F32, tag="to_s")   # [t | o] (bwd half reversed)
        v_s = pool.tile([SP, H * N], F16, tag="v_s")
        D_s = pool.tile([SP, H * N], F32, tag="D_s")
        M_s = pool.tile([SP, H * RS], F16, tag="M_s")    # rows 0..H-1
        Ml_s = pool.tile([SP, RS], F32, tag="Ml_s")      # row H, fp32
        ent_s = pool.tile([SP, N], F16, tag="ent_s")
        scr_s = pool.tile([SP, 1], F32, tag="scr_s")

        # dummy activation on scratch: hoists the Square table load (1.28us)
        # to t=0 so it overlaps the input DMAs instead of stalling row 1
        nc.scalar.activation(scr_s[:], scr_s[:],
                             ActivationFunctionType.Square)

        nc.sync.dma_start(to_s[:], to_d.ap())
        # eps*Omega chunks: first ones small so the build pipeline starts fast
        vch = [2, 2, 4, 8, 16, 32]
        r0 = 0
        for cn in vch:
            nc.sync.dma_start(
                ap(v_s, r0 * N, [[1, cn * N]]),
                bass.AP(v_d, r0 * N, [[H * N, SP], [1, cn * N]]))
            r0 += cn

        # M boundary: row 0 = BIG except M[0,0] = 0; col 0 of rows 1..H-1 =
        # BIG. The fp16 sentinel is 30000: far above any real path cost
        # (< ~1000) and never accumulated (every sentinel loses its min
        # against a real value within one step of the boundary).
        BIG16 = 30000.0
        nc.gpsimd.memset(ap(M_s, 0, [[1, RS]]), BIG16)
        nc.gpsimd.memset(ap(M_s, 0, [[1, 1]]), 0.0)
        nc.gpsimd.memset(ap(M_s, RS, [[RS, H - 1], [1, 1]]), BIG16)

        def m_off(r):
            return r * RS

        # D build: ACT fuses (t_i - o_j)^2 per row; +-eps*Omega is folded in
        # behind it. The first DVE_ADD_ROWS rows are added on DVE itself
        # via scan(bypass, add) (DVE is idle during startup and this keeps
        # Pool's sem hops off the early critical path); the rest go to Pool
        # as plain TT adds (TensorScalarPtr is DVE-only in the real ISA) in
        # small chunks — the DP consumes rows at ~321ns while ACT produces
        # at ~292ns and Pool adds ~254ns/row, so chunks must stay small
        # enough that chunk [a..b) completes before the DP reaches row a.
        DVE_ADD_ROWS = 7
        dve_adds = []
        bounds = [DVE_ADD_ROWS]
        while bounds[-1] < H:
            a = bounds[-1]
            cn = 2 if a < 13 else 4
            bounds.append(min(H, a + cn))
        o_ap = ap(to_s, N, [[1, N]])
        for i in range(DVE_ADD_ROWS):
            nc.scalar.activation(
                ap(D_s, i * N, [[1, N]]), o_ap,
                ActivationFunctionType.Square,
                bias=ap(to_s, i, [[1, 1]]), scale=-1.0)
            dch = ap(D_s, i * N, [[1, N]])
            dve_adds.append(nc.vector.tensor_tensor_scan(
                dch, dch, ap(v_s, i * N, [[1, N]]),
                0.0, AluOpType.bypass, AluOpType.add))
        for a, b in zip(bounds[:-1], bounds[1:]):
            for i in range(a, b):
                nc.scalar.activation(
                    ap(D_s, i * N, [[1, N]]), o_ap,
                    ActivationFunctionType.Square,
                    bias=ap(to_s, i, [[1, 1]]), scale=-1.0)
            dch = ap(D_s, a * N, [[1, (b - a) * N]])
            nc.gpsimd.tensor_tensor(
                dch, dch, ap(v_s, a * N, [[1, (b - a) * N]]),
                AluOpType.add)

        # forward min-plus DP: M[r,j] = D[r,j] + min(M[r-1,j-1], M[r-1,j], M[r,j-1])
        dp_insts = []
        for r in range(1, H + 1):
            dp_insts.append(nc.vector.tensor_tensor(
                ent_s[:],
                ap(M_s, m_off(r - 1), [[1, N]]),
                ap(M_s, m_off(r - 1) + 1, [[1, N]]),
                AluOpType.min))
            out_row = (ap(M_s, m_off(r) + 1, [[1, N]]) if r < H
                       else ap(Ml_s, 1, [[1, N]]))
            dp_insts.append(nc.vector.tensor_tensor_scan(
                out_row,
                ent_s[:],
                ap(D_s, (r - 1) * N, [[1, N]]),
                BIG, AluOpType.min, AluOpType.add))

        # The DP is one serial chain of same-engine (DVE) ops with
        # ascending same-shape access; engine in-order execution plus the
        # 128-cycle op length covers the SBUF write-ack pipeline, so the
        # intra-chain edges don't need runtime semaphores. Relax them to
        # nosync (queue-order) edges — without this every row pays two
        # ~90ns sem round-trips plus a SEQ-blocking EventSemaphore wait
        # (~290ns/row on the critical path).
        import bass_rust as _br
        _NOSYNC = _br.DependencyInfo.NO_SYNC_ONLY
        dp_insts = dve_adds + dp_insts
        dp_names = {bi.ins.name for bi in dp_insts}
        for bi in dp_insts:
            inst = bi.ins
            for name, info in inst.dependency_edges():
                if name in dp_names:
                    inst.remove_dependency(name)
                    inst.add_dependency(name, _NOSYNC)

        nc.sync.dma_start(vals_d.ap(), Ml_s[:])

    nc.compile()
    return nc


_NC_CACHE = None


def _get_nc():
    global _NC_CACHE
    if _NC_CACHE is None:
        _NC_CACHE = _build_kernel()
    return _NC_CACHE


def _v_const():
    """[SP, H*N] fp16 +-eps*Omega rows 1..H; Omega is reversal-invariant so
    the bwd half uses the same values. Sign per 32-block: [+, -, +, -]."""
    idx = np.arange(N, dtype=np.float64)
    om = ((idx[:, None] - idx[None, :]) ** 2)[:H].reshape(-1)
    v = (EPS * om).astype(np.float16)
    return np.concatenate([
        np.broadcast_to(v, (S, H * N)),
        np.broadcast_to(-v, (S, H * N)),
        np.broadcast_to(v, (S, H * N)),
        np.broadcast_to(-v, (S, H * N)),
    ]).astype(np.float16)


_EXEC_CACHE = None


def _get_exec():
    """Build the sharded jitted executable once (mirrors bass2jax's
    run_bass_via_pjrt multi-core path) and keep the constant v input
    resident on the devices."""
    global _EXEC_CACHE
    if _EXEC_CACHE is not None:
        return _EXEC_CACHE
    import jax
    import concourse.mybir as _mybir
    from jax.sharding import Mesh, PartitionSpec, NamedSharding
    from jax.experimental.shard_map import shard_map
    from concourse.bass2jax import (
        _bass_exec_p, install_neuronx_cc_hook, partition_id_tensor)

    nc = _get_nc()
    install_neuronx_cc_hook()
    partition_name = nc.partition_id_tensor.name if nc.partition_id_tensor else None
    in_names, out_names, out_avals, zero_outs = [], [], [], []
    for alloc in nc.m.functions[0].allocations:
        if not isinstance(alloc, _mybir.MemoryLocationSet):
            continue
        name = alloc.memorylocations[0].name
        if alloc.kind == "ExternalInput":
            if name != partition_name:
                in_names.append(name)
        elif alloc.kind == "ExternalOutput":
            shape = tuple(alloc.tensor_shape)
            dtype = _mybir.dt.np(alloc.dtype)
            out_names.append(name)
            out_avals.append(jax.core.ShapedArray(shape, dtype))
            zero_outs.append(np.zeros(shape, dtype))
    n_params = len(in_names)
    all_in_names = list(in_names) + list(out_names)
    if partition_name is not None:
        all_in_names.append(partition_name)
    donate = tuple(range(n_params, n_params + len(out_names)))

    def _body(*args):
        operands = list(args)
        if partition_name is not None:
            operands.append(partition_id_tensor())
        return tuple(_bass_exec_p.bind(
            *operands,
            out_avals=tuple(out_avals),
            in_names=tuple(all_in_names),
            out_names=tuple(out_names),
            lowering_input_output_aliases=(),
            sim_require_finite=True,
            sim_require_nnan=True,
            nc=nc,
        ))

    devices = jax.devices()[:N_CORES]
    mesh = Mesh(np.asarray(devices), ("core",))
    in_specs = (PartitionSpec("core"),) * (n_params + len(out_names))
    out_specs = (PartitionSpec("core"),) * len(out_names)
    sharded = jax.jit(
        shard_map(_body, mesh=mesh, in_specs=in_specs, out_specs=out_specs,
                  check_rep=False),
        donate_argnums=donate, keep_unused=True)
    shard = NamedSharding(mesh, PartitionSpec("core"))
    v_dev = jax.device_put(
        np.concatenate([_v_const()] * N_CORES, axis=0), shard)
    _EXEC_CACHE = (sharded, in_names, out_names, zero_outs, shard, v_dev)
    return _EXEC_CACHE


def kernel(outputs, targets):
    """outputs, targets: [64, 128, 4] float32 -> scalar float32 loss."""
    sharded, in_names, out_names, zero_outs, shard, v_dev = _get_exec()
    outputs = np.asarray(outputs, np.float32)
    targets = np.asarray(targets, np.float32)
    B, T, C = outputs.shape
    t = np.transpose(targets, (0, 2, 1)).reshape(N_CORES, S, T)
    o = np.transpose(outputs, (0, 2, 1)).reshape(N_CORES, S, T)
    # per core, 128 partitions: [fwd A | fwd B | bwd A | bwd B];
    # free dim = [t | o], with the bwd halves index-reversed
    fwd = np.concatenate([t, o], axis=2)                        # [8, 32, 2T]
    bwd = np.concatenate([t[:, :, ::-1], o[:, :, ::-1]], axis=2)
    to = np.concatenate([fwd, fwd, bwd, bwd], axis=1)           # [8, 128, 2T]
    to2 = np.ascontiguousarray(to.reshape(N_CORES * SP, 2 * T))
    by_name = {"to": to2, "v": v_dev}
    concat_in = [by_name[name] for name in in_names]
    concat_zeros = [
        np.zeros((N_CORES * z.shape[0], *z.shape[1:]), z.dtype) for z in zero_outs
    ]
    out_arrs = sharded(*concat_in, *concat_zeros)
    outs = {name: np.asarray(out_arrs[i]) for i, name in enumerate(out_names)}
    rows = outs["vals"].reshape(N_CORES, 4, S, RS).astype(np.float64)
    # meeting-point merge between rows H and H+1:
    #   total = min_j ( Mf[H,j] + min(num[H+1,j], num[H+1,j+1]) )
    #   num[H+1,j] = Br[H, N+1-j]
    j = np.arange(1, N + 1)
    BIGF = np.float64(BIG)
    res = {}
    for h, (fi, bi) in enumerate(((0, 2), (1, 3))):
        Mf, Br = rows[:, fi], rows[:, bi]                       # [8, 32, RS]
        numj = Br[:, :, N + 1 - j]
        numj1 = np.where((N - j) >= 1, Br[:, :, np.clip(N - j, 0, N)], BIGF)
        res[h] = (Mf[:, :, 1:] + np.minimum(numj, numj1)).min(axis=2)
    A, Bm = res[0], res[1]
    vals = (A + Bm) / 2.0
    s_fd = (A - Bm) / (2.0 * EPS)
    loss = 0.5 * (vals.sum() / B) + 0.5 * (s_fd.sum() / (B * T * T))
    return np.float32(loss)


# revision 35
# speedup vs baseline: 1.0373x; 1.0124x over previous
"""DILATE loss (soft-DTW shape + temporal) on 8 Trainium2 NeuronCores.

Strategy: central finite difference + bidirectional DP.

gamma=0.01 makes the soft-DTW effectively a hard min-plus (Viterbi) DP,
and the temporal term is
  sum(E * Omega) = d/d(eps) softdtw(D + eps*Omega)  at eps=0
since E = d(softdtw)/dD. Each series is solved on D + eps*Omega and on
D - eps*Omega and the host combines:
  vals = (A + B) / 2                  (shape term)
  sum(E*Omega) = (A - B) / (2 eps)    (temporal term)

The DP itself is split at the middle row: a forward chain from (0,0)
and a suffix chain from (N,N) — the latter is the same forward
recurrence on the index-reversed cost matrix, and Omega is
reversal-invariant. Both chains for both FD signs run in ONE
64-row instruction stream over 128 SBUF partitions
(32 series x {+eps,-eps} x {fwd,bwd}); per-op cost on trn2 depends
only on free-dim size, so the extra partitions are time-free while the
serial DP halves. The halves meet between rows 64 and 65:
  total = min_j ( Mf[64,j] + min(num[65,j], num[65,j+1]) )
with num[65,j] = Br[64, N+1-j]; this 128-element reduction joins the
scalar loss reduction on the host.

Engines: ACT builds D rows fused as Square(-o_j + t_i) (bias = t_i per
partition); Pool folds +-eps*Omega in behind it; DVE runs the serial
chain 64 x (TT-min fp16-2x + scan(min,add)). The intra-chain DVE edges
are relaxed to nosync (queue-order) so rows chain back-to-back. M rows
are stored fp16 (scan state is fp32 internally); the final row of each
chain is written fp32 so the corner values the FD differences keep full
precision.
"""
import sys
if "/opt/trn_rl_repo" not in sys.path:
    sys.path.insert(0, "/opt/trn_rl_repo")
import numpy as np
from contextlib import ExitStack

import concourse.bass as bass
import concourse.bacc as bacc
import concourse.mybir as mybir
import concourse.tile as tile
from concourse.mybir import AluOpType, ActivationFunctionType

F32 = mybir.dt.float32
F16 = mybir.dt.float16
S = 32          # series per core
SP = 128        # partitions: series x {+eps,-eps} x {fwd,bwd}
N = 128         # DP size (= T)
H = N // 2      # rows per chain
RS = N + 1      # M-table row stride (col 0 = boundary)
BIG = 1e30
EPS = 3e-5      # FD step on the Omega perturbation
N_CORES = 8


def ap(t, off, dims):
    base = t[:]
    return bass.AP(base.tensor, base.offset + off, [base.ap[0]] + dims)


def _build_kernel():
    nc = bacc.Bacc("TRN2", target_bir_lowering=False, debug=False)
    to_d = nc.dram_tensor("to", [SP, 2 * N], F32, kind="ExternalInput")
    v_d = nc.dram_tensor("v", [SP, H * N], F16, kind="ExternalInput")
    vals_d = nc.dram_tensor("vals", [SP, RS], F32, kind="ExternalOutput")

    with tile.TileContext(nc) as tc, ExitStack() as ctx:
        pool = ctx.enter_context(tc.tile_pool(name="main", bufs=1))
        to_s = pool.tile([SP, 2 * N], F32, tag="to_s")   # [t | o] (bwd half reversed)
        v_s = pool.tile([SP, H * N], F16, tag="v_s")
        D_s = pool.tile([SP, H * N], F16, tag="D_s")
        M_s = pool.tile([SP, H * RS], F16, tag="M_s")    # rows 0..H-1
        Ml_s = pool.tile([SP, RS], F32, tag="Ml_s")      # row H, fp32
        ent_s = pool.tile([SP, N], F16, tag="ent_s")
        scr_s = pool.tile([SP, 1], F32, tag="scr_s")

        # dummy activation on scratch: hoists the Square table load (1.28us)
        # to t=0 so it overlaps the input DMAs instead of stalling row 1
        nc.scalar.activation(scr_s[:], scr_s[:],
                             ActivationFunctionType.Square)

        nc.sync.dma_start(to_s[:], to_d.ap())
        # eps*Omega chunks: first ones small so the build pipeline starts fast
        vch = [2, 2, 4, 8, 16, 32]
        r0 = 0
        for cn in vch:
            nc.sync.dma_start(
                ap(v_s, r0 * N, [[1, cn * N]]),
                bass.AP(v_d, r0 * N, [[H * N, SP], [1, cn * N]]))
            r0 += cn

        # M boundary: row 0 = BIG except M[0,0] = 0; col 0 of rows 1..H-1 =
        # BIG. The fp16 sentinel is 30000: far above any real path cost
        # (< ~1000) and never accumulated (every sentinel loses its min
        # against a real value within one step of the boundary).
        BIG16 = 30000.0
        nc.gpsimd.memset(ap(M_s, 0, [[1, RS]]), BIG16)
        nc.gpsimd.memset(ap(M_s, 0, [[1, 1]]), 0.0)
        nc.gpsimd.memset(ap(M_s, RS, [[RS, H - 1], [1, 1]]), BIG16)

        def m_off(r):
            return r * RS

        # D build: ACT fuses (t_i - o_j)^2 per row; +-eps*Omega is folded in
        # behind it. The first DVE_ADD_ROWS rows are added on DVE itself
        # via scan(bypass, add) (DVE is idle during startup and this keeps
        # Pool's sem hops off the early critical path); the rest go to Pool
        # as plain TT adds (TensorScalarPtr is DVE-only in the real ISA) in
        # small chunks — the DP consumes rows at ~321ns while ACT produces
        # at ~292ns and Pool adds ~254ns/row, so chunks must stay small
        # enough that chunk [a..b) completes before the DP reaches row a.
        DVE_ADD_ROWS = 7
        dve_adds = []
        bounds = [DVE_ADD_ROWS]
        while bounds[-1] < H:
            a = bounds[-1]
            cn = 2 if a < 13 else 4
            bounds.append(min(H, a + cn))
        o_ap = ap(to_s, N, [[1, N]])
        for i in range(DVE_ADD_ROWS):
            nc.scalar.activation(
                ap(D_s, i * N, [[1, N]]), o_ap,
                ActivationFunctionType.Square,
                bias=ap(to_s, i, [[1, 1]]), scale=-1.0)
            dch = ap(D_s, i * N, [[1, N]])
            dve_adds.append(nc.vector.tensor_tensor(
                dch, dch, ap(v_s, i * N, [[1, N]]),
                AluOpType.add))
        for a, b in zip(bounds[:-1], bounds[1:]):
            for i in range(a, b):
                nc.scalar.activation(
                    ap(D_s, i * N, [[1, N]]), o_ap,
                    ActivationFunctionType.Square,
                    bias=ap(to_s, i, [[1, 1]]), scale=-1.0)
            dch = ap(D_s, a * N, [[1, (b - a) * N]])
            nc.gpsimd.tensor_tensor(
                dch, dch, ap(v_s, a * N, [[1, (b - a) * N]]),
                AluOpType.add)

        # forward min-plus DP: M[r,j] = D[r,j] + min(M[r-1,j-1], M[r-1,j], M[r,j-1])
        dp_insts = []
        for r in range(1, H + 1):
            dp_insts.append(nc.vector.tensor_tensor(
                ent_s[:],
                ap(M_s, m_off(r - 1), [[1, N]]),
                ap(M_s, m_off(r - 1) + 1, [[1, N]]),
                AluOpType.min))
            out_row = (ap(M_s, m_off(r) + 1, [[1, N]]) if r < H
                       else ap(Ml_s, 1, [[1, N]]))
            dp_insts.append(nc.vector.tensor_tensor_scan(
                out_row,
                ent_s[:],
                ap(D_s, (r - 1) * N, [[1, N]]),
                BIG, AluOpType.min, AluOpType.add))

        # The DP is one serial chain of same-engine (DVE) ops with
        # ascending same-shape access; engine in-order execution plus the
        # 128-cycle op length covers the SBUF write-ack pipeline, so the
        # intra-chain edges don't need runtime semaphores. Relax them to
        # nosync (queue-order) edges — without this every row pays two
        # ~90ns sem round-trips plus a SEQ-blocking EventSemaphore wait
        # (~290ns/row on the critical path).
        import bass_rust as _br
        _NOSYNC = _br.DependencyInfo.NO_SYNC_ONLY
        dp_insts = dve_adds + dp_insts
        dp_names = {bi.ins.name for bi in dp_insts}
        for bi in dp_insts:
            inst = bi.ins
            for name, info in inst.dependency_edges():
                if name in dp_names:
                    inst.remove_dependency(name)
                    inst.add_dependency(name, _NOSYNC)

        nc.sync.dma_start(vals_d.ap(), Ml_s[:])

    nc.compile()
    return nc


_NC_CACHE = None


def _get_nc():
    global _NC_CACHE
    if _NC_CACHE is None:
        _NC_CACHE = _build_kernel()
    return _NC_CACHE


def _v_const():
    """[SP, H*N] fp16 +-eps*Omega rows 1..H; Omega is reversal-invariant so
    the bwd half uses the same values. Sign per 32-block: [+, -, +, -]."""
    idx = np.arange(N, dtype=np.float64)
    om = ((idx[:, None] - idx[None, :]) ** 2)[:H].reshape(-1)
    v = (EPS * om).astype(np.float16)
    return np.concatenate([
        np.broadcast_to(v, (S, H * N)),
        np.broadcast_to(-v, (S, H * N)),
        np.broadcast_to(v, (S, H * N)),
        np.broadcast_to(-v, (S, H * N)),
    ]).astype(np.float16)


_EXEC_CACHE = None


def _get_exec():
    """Build the sharded jitted executable once (mirrors bass2jax's
    run_bass_via_pjrt multi-core path) and keep the constant v input
    resident on the devices."""
    global _EXEC_CACHE
    if _EXEC_CACHE is not None:
        return _EXEC_CACHE
    import jax
    import concourse.mybir as _mybir
    from jax.sharding import Mesh, PartitionSpec, NamedSharding
    from jax.experimental.shard_map import shard_map
    from concourse.bass2jax import (
        _bass_exec_p, install_neuronx_cc_hook, partition_id_tensor)

    nc = _get_nc()
    install_neuronx_cc_hook()
    partition_name = nc.partition_id_tensor.name if nc.partition_id_tensor else None
    in_names, out_names, out_avals, zero_outs = [], [], [], []
    for alloc in nc.m.functions[0].allocations:
        if not isinstance(alloc, _mybir.MemoryLocationSet):
            continue
        name = alloc.memorylocations[0].name
        if alloc.kind == "ExternalInput":
            if name != partition_name:
                in_names.append(name)
        elif alloc.kind == "ExternalOutput":
            shape = tuple(alloc.tensor_shape)
            dtype = _mybir.dt.np(alloc.dtype)
            out_names.append(name)
            out_avals.append(jax.core.ShapedArray(shape, dtype))
            zero_outs.append(np.zeros(shape, dtype))
    n_params = len(in_names)
    all_in_names = list(in_names) + list(out_names)
    if partition_name is not None:
        all_in_names.append(partition_name)
    donate = tuple(range(n_params, n_params + len(out_names)))

    def _body(*args):
        operands = list(args)
        if partition_name is not None:
            operands.append(partition_id_tensor())
        return tuple(_bass_exec_p.bind(
            *operands,
            out_avals=tuple(out_avals),
            in_names=tuple(all_in_names),
            out_names=tuple(out_names),
            lowering_input_output_aliases=(),
            sim_require_finite=True,
            sim_require_nnan=True,
            nc=nc,
        ))

    devices = jax.devices()[:N_CORES]
    mesh = Mesh(np.asarray(devices), ("core",))
    in_specs = (PartitionSpec("core"),) * (n_params + len(out_names))
    out_specs = (PartitionSpec("core"),) * len(out_names)
    sharded = jax.jit(
        shard_map(_body, mesh=mesh, in_specs=in_specs, out_specs=out_specs,
                  check_rep=False),
        donate_argnums=donate, keep_unused=True)
    shard = NamedSharding(mesh, PartitionSpec("core"))
    v_dev = jax.device_put(
        np.concatenate([_v_const()] * N_CORES, axis=0), shard)
    _EXEC_CACHE = (sharded, in_names, out_names, zero_outs, shard, v_dev)
    return _EXEC_CACHE


def kernel(outputs, targets):
    """outputs, targets: [64, 128, 4] float32 -> scalar float32 loss."""
    sharded, in_names, out_names, zero_outs, shard, v_dev = _get_exec()
    outputs = np.asarray(outputs, np.float32)
    targets = np.asarray(targets, np.float32)
    B, T, C = outputs.shape
    t = np.transpose(targets, (0, 2, 1)).reshape(N_CORES, S, T)
    o = np.transpose(outputs, (0, 2, 1)).reshape(N_CORES, S, T)
    # per core, 128 partitions: [fwd A | fwd B | bwd A | bwd B];
    # free dim = [t | o], with the bwd halves index-reversed
    fwd = np.concatenate([t, o], axis=2)                        # [8, 32, 2T]
    bwd = np.concatenate([t[:, :, ::-1], o[:, :, ::-1]], axis=2)
    to = np.concatenate([fwd, fwd, bwd, bwd], axis=1)           # [8, 128, 2T]
    to2 = np.ascontiguousarray(to.reshape(N_CORES * SP, 2 * T))
    by_name = {"to": to2, "v": v_dev}
    concat_in = [by_name[name] for name in in_names]
    concat_zeros = [
        np.zeros((N_CORES * z.shape[0], *z.shape[1:]), z.dtype) for z in zero_outs
    ]
    out_arrs = sharded(*concat_in, *concat_zeros)
    outs = {name: np.asarray(out_arrs[i]) for i, name in enumerate(out_names)}
    rows = outs["vals"].reshape(N_CORES, 4, S, RS).astype(np.float64)
    # meeting-point merge between rows H and H+1:
    #   total = min_j ( Mf[H,j] + min(num[H+1,j], num[H+1,j+1]) )
    #   num[H+1,j] = Br[H, N+1-j]
    j = np.arange(1, N + 1)
    BIGF = np.float64(BIG)
    res = {}
    for h, (fi, bi) in enumerate(((0, 2), (1, 3))):
        Mf, Br = rows[:, fi], rows[:, bi]                       # [8, 32, RS]
        numj = Br[:, :, N + 1 - j]
        numj1 = np.where((N - j) >= 1, Br[:, :, np.clip(N - j, 0, N)], BIGF)
        res[h] = (Mf[:, :, 1:] + np.minimum(numj, numj1)).min(axis=2)
    A, Bm = res[0], res[1]
    vals = (A + Bm) / 2.0
    s_fd = (A - Bm) / (2.0 * EPS)
    loss = 0.5 * (vals.sum() / B) + 0.5 * (s_fd.sum() / (B * T * T))
    return np.float32(loss)
